# revision 1
# baseline (speedup 1.0000x reference)
"""Sparse dual-masked attention for Trainium2, 8 NeuronCores.

Problem: B=2, N=2048, DIM=512, H=8, DH=64.
  qkv = x @ W_qkv; per-head attention with dual mask
  (np_i*np_j==0 | bert_j==1 -> -1000), softmax, out proj + bias.

Key structure exploited (sparse_attention):
  - A row i with np_i==0 is fully masked -> softmax is uniform -> output row
    is the constant mean(V) @ W_out + b_out (computed on host; tiny).
  - For rows with np_i==1, only columns with np_j==1 & bert_j==0 survive
    (exp(-1000-max) == 0 exactly in the reference). So we gather those
    ~R=1030 rows and ~M=535 columns on the host and run a dense attention
    over the gathered set on device: ~8x less work than dense.

Sharding: core = (batch b, head-pair g): 2 batches x 4 head groups.
  W_qkv is split column-wise per head pair, W_out row-wise; each core
  produces a partial [R,512] output; host sums the 4 partials per batch.

Device dataflow per core (R=R_PAD rows, M=M_PAD kv cols, 2 heads):
  x shipped pre-transposed/gathered as xT [512, R] (kv rows first, a zero
  gap up to M_PAD, then the remaining attending rows); kvc [128, NMT] is
  the kv-indicator column per m-tile, written into V's ones-columns to
  produce the softmax denominators for free during attn @ V.
  1. Q^T = (0.125*Wq)^T x^T   [128, R]   (tensor engine, PSUM accum)
     K^T = Wk^T x^T           [128, M]
     V_aug = x^T^T @ Wv_aug   [M, 130]   (per m-tile; cols Vh0|kv1|Vh1|kv1)
  2. S^T[h] = K_h Q_h^T       [m-tile 128, R]  (contraction d=64; the two
     heads' matmuls sit on disjoint PE row groups and overlap)
     P^T = exp(S^T)           (ScalarE, PSUM->SBUF)
  3. O^T[h] = V_aug_h^T P^T   [65, R]  accumulated over m-tiles; row 64 is
     the softmax denominator (via the kvc column).
  4. recip = 1/denoms (DVE approx, ~51 ulp), replicated across 64
     partitions by a rank-1 matmul; O^T normalized by tensor_mul.
  5. y = O_norm^T^T @ W_out_rows  [R, 512] -> DMA out (host adds bias),
     interleaved with phase 3 so output DMAs overlap compute.
"""

import numpy as np

_CORES = 8
S_F32R = False  # float32r on Q/K/logits would be ~4x faster on those
                # matmuls but costs ~1e-4 scale-relative error; the
                # correctness gate is tight, so stay full fp32.
_DIM = 512
_DH = 64
_H = 8
_INNER = _H * _DH


def _ceil_to(x, m):
    return ((x + m - 1) // m) * m


def _chunks(total, step):
    out = []
    o = 0
    while o < total:
        out.append((o, min(step, total - o)))
        o += step
    return out


def _chunks_ge(total, step=512, minc=256):
    """Chunks of <= step, each >= minc (rebalancing the tail)."""
    out = _chunks(total, step)
    if len(out) >= 2 and out[-1][1] < minc:
        o_prev, w_prev = out[-2]
        o_last, w_last = out[-1]
        move = minc - w_last
        out[-2] = (o_prev, w_prev - move)
        out[-1] = (o_last - move, w_last + move)
    return out


def build_bass(R_PAD, M_PAD):
    """Build the SPMD bass program for padded sizes R_PAD (queries) and
    M_PAD (kv columns). Returns the compiled Bacc object.

    All matmuls run in full float32 (4 cyc/row on the PE; fp32 matmuls
    lower to LOW/HIGH pass pairs). float32r (1 cyc/row at free-dim >= 256)
    was measured ~4x faster per matmul but its ~2^-12 operand/product
    rounding costs ~1e-4 scale-relative output error vs the ~5e-7
    intrinsic fp32 envelope of this computation -- too risky against the
    absmax gate, and end-to-end it only saved a few us (see S_F32R)."""
    import concourse.bacc as bacc
    import concourse.mybir as mybir
    import concourse.tile as tile

    f32 = mybir.dt.float32
    f32r = mybir.dt.float32r if S_F32R else mybir.dt.float32
    EXP = mybir.ActivationFunctionType.Exp

    assert R_PAD % 16 == 0 and M_PAD % 128 == 0 and R_PAD >= M_PAD
    NMT = M_PAD // 128          # kv m-tiles
    NRT = (R_PAD + 127) // 128  # query r-tiles for the final projection
    RC = _chunks_ge(R_PAD)      # chunks >= 256 amortize per-matmul overhead
    MC = _chunks_ge(M_PAD)
    assert len(RC) <= 3         # denominator rows live at partitions 0/32/64

    nc = bacc.Bacc("TRN2", target_bir_lowering=False, debug=False,
                   num_devices=_CORES)

    xT_d = nc.dram_tensor("xT", [512, R_PAD], f32r, kind="ExternalInput")
    wq_d = nc.dram_tensor("wq", [512, 128], f32r, kind="ExternalInput")
    wk_d = nc.dram_tensor("wk", [512, 128], f32r, kind="ExternalInput")
    wv_d = nc.dram_tensor("wv", [512, 128], f32, kind="ExternalInput")
    kvc_d = nc.dram_tensor("kvc", [128, NMT], f32, kind="ExternalInput")
    wo_d = nc.dram_tensor("wo", [128, 512], f32, kind="ExternalInput")
    y_d = nc.dram_tensor("y", [R_PAD, 512], f32, kind="ExternalOutput")

    with tile.TileContext(nc) as tc:
        with (
            tc.tile_pool(name="consts", bufs=1) as consts,
            tc.tile_pool(name="pt", bufs=2 * NMT) as ptpool,
            tc.tile_pool(name="ysb", bufs=5) as ypool,
            tc.tile_pool(name="rcp", bufs=4) as rpool,
            tc.tile_pool(name="pbig", bufs=4, space="PSUM") as pbig,
            tc.tile_pool(name="po", bufs=3, space="PSUM") as po,
            tc.tile_pool(name="prep", bufs=1, space="PSUM") as prep,
        ):
            # ---- input DMAs: issue split across engines so the first
            # compute inputs (wq, xT chunk 0) complete first ----------------
            wq = consts.tile([128, 4, 128], f32r, tag="wq")
            nc.sync.dma_start(
                out=wq, in_=wq_d.ap().rearrange("(a p) d -> p a d", p=128))
            xT = consts.tile([128, 4, R_PAD], f32r, tag="xT")
            xeng = [nc.scalar, nc.gpsimd, nc.scalar, nc.sync]
            for c in range(4):
                xeng[c].dma_start(
                    out=xT[:, c, :], in_=xT_d.ap()[c * 128:(c + 1) * 128, :])
            wk = consts.tile([128, 4, 128], f32r, tag="wk")
            nc.sync.dma_start(
                out=wk, in_=wk_d.ap().rearrange("(a p) d -> p a d", p=128))
            wv = consts.tile([128, 4, 128], f32, tag="wv")
            nc.gpsimd.dma_start(
                out=wv, in_=wv_d.ap().rearrange("(a p) d -> p a d", p=128))
            kvc = consts.tile([128, NMT], f32, tag="kvc")
            nc.gpsimd.dma_start(out=kvc, in_=kvc_d.ap())
            wo = consts.tile([128, 512], f32, tag="wo")
            nc.gpsimd.dma_start(out=wo, in_=wo_d.ap())

            # ---- phase 1: projections --------------------------------------
            ones = consts.tile([1, 64], f32, tag="ones")
            nc.vector.memset(ones, 1.0)

            QT = consts.tile([128, R_PAD], f32r, tag="QT")
            qps = [pbig.tile([128, 512], f32, tag="big", name=f"qps{i}")
                   for i in range(len(RC))]
            for c in range(4):
                for i, (o, w) in enumerate(RC):
                    nc.tensor.matmul(qps[i][:, :w], wq[:, c, :],
                                     xT[:, c, o:o + w],
                                     start=(c == 0), stop=(c == 3))
            for i, (o, w) in enumerate(RC):
                nc.scalar.copy(QT[:, o:o + w], qps[i][:, :w])

            KT = consts.tile([128, M_PAD], f32r, tag="KT")
            kps = [pbig.tile([128, 512], f32, tag="big", name=f"kps{i}")
                   for i in range(len(MC))]
            for c in range(4):
                for i, (o, w) in enumerate(MC):
                    nc.tensor.matmul(kps[i][:, :w], wk[:, c, :],
                                     xT[:, c, o:o + w],
                                     start=(c == 0), stop=(c == 3))
            for i, (o, w) in enumerate(MC):
                nc.scalar.copy(KT[:, o:o + w], kps[i][:, :w])

            V = []
            for mt in range(NMT):
                ps = pbig.tile([128, 512], f32, tag="big")
                sl = slice(mt * 128, (mt + 1) * 128)
                for c in range(4):
                    nc.tensor.matmul(ps[:, :128], xT[:, c, sl].bitcast(f32) if S_F32R else xT[:, c, sl],
                                     wv[:, c, :], start=(c == 0), stop=(c == 3))
                # V_aug layout per head: [kv1 | pad | V(64) at cols 64:128]
                # so the attn@V output carries the softmax denominator at
                # partition 0 (custom-DVE recip needs base 0) and O at the
                # 64-aligned partitions 64:128. Rows are scaled by the kv
                # indicator to null tail rows sitting below M_PAD.
                vt = consts.tile([128, 256], f32, tag=f"v{mt}", name=f"v{mt}")
                nc.vector.memset(vt, 0.0)
                nc.vector.tensor_scalar_mul(vt[:, 64:128], in0=ps[:, 0:64],
                                            scalar1=kvc[:, mt:mt + 1])
                nc.vector.tensor_scalar_mul(vt[:, 192:256], in0=ps[:, 64:128],
                                            scalar1=kvc[:, mt:mt + 1])
                nc.vector.tensor_copy(vt[:, 0:1], kvc[:, mt:mt + 1])
                nc.vector.tensor_copy(vt[:, 128:129], kvc[:, mt:mt + 1])
                V.append(vt)

            # ---- phase 2: S^T + exp (heads adjacent: PE row-group overlap) -
            PT = {}
            for h in range(2):
                for mt in range(NMT):
                    PT[(h, mt)] = ptpool.tile([128, R_PAD], f32, tag="pt",
                                              name=f"pt{h}_{mt}")
            for mt in range(NMT):
                msl = slice(mt * 128, (mt + 1) * 128)
                for (o, w) in RC:
                    pss = []
                    for h in range(2):
                        ps = pbig.tile([128, 512], f32, tag="big")
                        pss.append(ps)
                        hs = slice(h * 64, (h + 1) * 64)
                        nc.tensor.matmul(ps[:, :w], KT[hs, msl],
                                         QT[hs, o:o + w], start=True, stop=True)
                    for h in range(2):
                        nc.scalar.activation(out=PT[(h, mt)][:, o:o + w],
                                             in_=pss[h][:, :w], func=EXP)

            # ---- phase 3: O^T, denominators, normalize; y proj interleaved -
            OnT = consts.tile([128, R_PAD], f32, tag="OnT")
            ydone = 0
            for h in range(2):
                vs = slice(h * 128, (h + 1) * 128)
                for i, (o, w) in enumerate(RC):
                    ops = po.tile([128, 512], f32, tag="o")
                    for mt in range(NMT):
                        nc.tensor.matmul(ops[:, :w], V[mt][:, vs],
                                         PT[(h, mt)][:, o:o + w],
                                         start=(mt == 0), stop=(mt == NMT - 1))
                    rcp = rpool.tile([1, 512], f32, tag="rcp")
                    nc.vector.reciprocal_approx_fast(rcp[:, :w], ops[0:1, :w])
                    rep = prep.tile([64, 512], f32, tag="rep")
                    nc.tensor.matmul(rep[:, :w], ones[0:1, :], rcp[:, :w],
                                     start=True, stop=True)
                    rep_sb = rpool.tile([64, 512], f32, tag="repsb")
                    if i % 2 == 0:
                        nc.scalar.copy(rep_sb[:, :w], rep[:, :w])
                    else:
                        nc.vector.tensor_copy(rep_sb[:, :w], rep[:, :w])
                    nc.vector.tensor_mul(OnT[h * 64:(h + 1) * 64, o:o + w],
                                         ops[64:128, :w], rep_sb[:, :w])
                    if h == 1:
                        # phase 4: out projection for the r-tiles fully
                        # covered so far (both heads normalized)
                        done = o + w
                        while ydone < NRT and min(ydone * 128 + 128,
                                                   R_PAD) <= done:
                            rt = ydone
                            tw = min(128, R_PAD - rt * 128)
                            ps = pbig.tile([128, 512], f32, tag="big")
                            rsl = slice(rt * 128, rt * 128 + tw)
                            nc.tensor.matmul(ps[:tw, :], OnT[:, rsl], wo,
                                             start=True, stop=True)
                            ysb = ypool.tile([128, 512], f32, tag="y")
                            if rt % 2 == 0:
                                nc.scalar.copy(ysb[:tw, :], ps[:tw, :])
                            else:
                                nc.vector.tensor_copy(ysb[:tw, :], ps[:tw, :])
                            nc.default_dma_engine.dma_start(
                                out=y_d.ap()[rsl, :], in_=ysb[:tw, :])
                            ydone += 1

    nc.compile()
    return nc


def _prep(x, mask_np, mask_bert, W_qkv, W_out):
    """Host-side gather/shard. Returns (in_maps, meta)."""
    B, N, DIM = x.shape
    assert (B, DIM) == (2, _DIM)
    x = np.ascontiguousarray(x, dtype=np.float32)
    W_qkv = np.ascontiguousarray(W_qkv, dtype=np.float32)
    W_out = np.ascontiguousarray(W_out, dtype=np.float32)

    kv_idx, tail_idx, Ms, tails = [], [], [], []
    for b in range(B):
        npb = mask_np[b].astype(bool)
        bb = mask_bert[b].astype(bool)
        kv = np.nonzero(npb & ~bb)[0]
        tl = np.nonzero(npb & bb)[0]
        kv_idx.append(kv)
        tail_idx.append(tl)
        Ms.append(len(kv))
        tails.append(len(tl))

    M_PAD = max(128, _ceil_to(max(Ms), 128))
    # rows are packed [kv | tail] with no gap: the tail rows that fall in
    # [M_b, M_PAD) act as key/value candidates but are nulled by the kvc
    # indicator (V rows scaled to 0, denominator column 0), so no zero gap
    # is needed and R_PAD shrinks to the real row count.
    R_PAD = max(128, _ceil_to(max(Ms[b] + tails[b] for b in range(B)), 16),
                M_PAD)

    NMT = M_PAD // 128
    xT_b, kvc_b, row_pos = [], [], []
    for b in range(B):
        xa = np.zeros((512, R_PAD), dtype=np.float32)
        xa[:, :Ms[b]] = x[b][kv_idx[b]].T
        xa[:, Ms[b]:Ms[b] + tails[b]] = x[b][tail_idx[b]].T
        xT_b.append(xa)
        kvones = np.zeros(M_PAD, dtype=np.float32)
        kvones[:Ms[b]] = 1.0
        kvc_b.append(np.ascontiguousarray(kvones.reshape(NMT, 128).T))
        # output row p of the device result corresponds to token row_pos[p]
        pos = np.concatenate([kv_idx[b], tail_idx[b]])
        row_pos.append(pos)

    scale = np.float32(_DH ** -0.5)
    in_maps = []
    for c in range(_CORES):
        b, g = divmod(c, 4)
        qc = slice(128 * g, 128 * g + 128)
        kc = slice(_INNER + 128 * g, _INNER + 128 * g + 128)
        vc = slice(2 * _INNER + 128 * g, 2 * _INNER + 128 * g + 128)
        wq = np.ascontiguousarray(W_qkv[:, qc] * scale)
        wk = np.ascontiguousarray(W_qkv[:, kc])
        wv = np.ascontiguousarray(W_qkv[:, vc])
        wo = np.ascontiguousarray(W_out[128 * g:128 * g + 128, :])
        in_maps.append({"xT": xT_b[b], "wq": wq, "wk": wk, "wv": wv, "wo": wo,
                        "kvc": kvc_b[b]})

    meta = dict(M_PAD=M_PAD, R_PAD=R_PAD, Ms=Ms, tails=tails,
                kv_idx=kv_idx, tail_idx=tail_idx, row_pos=row_pos)
    return in_maps, meta


def _assemble(results, meta, x, mask_np, W_qkv, W_out, b_out):
    B, N, _ = x.shape
    M_PAD = meta["M_PAD"]
    out = np.empty((B, N, _DIM), dtype=np.float32)
    Wv_full = W_qkv[:, 2 * _INNER:].astype(np.float32)
    for b in range(B):
        # constant output for fully-masked rows: uniform attention = mean(V)
        meanv = (x[b].mean(axis=0, dtype=np.float32) @ Wv_full)
        yconst = meanv @ W_out.astype(np.float32) + b_out
        out[b, :, :] = yconst[None, :]
        Mb, tb = meta["Ms"][b], meta["tails"][b]
        if Mb == 0:
            # no unmasked kv columns: every row is fully masked -> uniform
            continue
        acc = None
        for g in range(4):
            yp = results[4 * b + g]["y"]
            acc = yp.copy() if acc is None else acc + yp
        out[b, meta["row_pos"][b], :] = acc[:Mb + tb] + b_out
    return out


_CACHE = {}


def _get_bass(R_PAD, M_PAD):
    key = (R_PAD, M_PAD, S_F32R)
    if key not in _CACHE:
        _CACHE[key] = build_bass(R_PAD, M_PAD)
    return _CACHE[key]


def run_spmd(in_maps, meta, trace=False, tmpdir=None, trace_cores=None):
    from concourse.bass_utils import run_bass_kernel_spmd

    nc = _get_bass(meta["R_PAD"], meta["M_PAD"])
    return run_bass_kernel_spmd(
        nc, in_maps, core_ids=list(range(_CORES)), trace=trace, tmpdir=tmpdir,
        trace_cores=trace_cores)


def kernel(x, mask_np, mask_bert, W_qkv, W_out, b_out):
    x = np.asarray(x)
    mask_np = np.asarray(mask_np)
    mask_bert = np.asarray(mask_bert)
    W_qkv = np.asarray(W_qkv, dtype=np.float32)
    W_out = np.asarray(W_out, dtype=np.float32)
    b_out = np.asarray(b_out, dtype=np.float32)

    in_maps, meta = _prep(x, mask_np, mask_bert, W_qkv, W_out)
    res = run_spmd(in_maps, meta)
    return _assemble(res.results, meta, x, mask_np, W_qkv, W_out, b_out)



# revision 10
# speedup vs baseline: 1.5663x; 1.5663x over previous
"""Sparse dual-masked attention for Trainium2, 8 NeuronCores.

Problem: B=2, N=2048, DIM=512, H=8, DH=64.
  qkv = x @ W_qkv; per-head attention with dual mask
  (np_i*np_j==0 | bert_j==1 -> -1000), softmax, out proj + bias.

Structure exploited (sparse_attention):
  - A row i with np_i==0 is fully masked -> softmax uniform -> output row is
    the constant mean(V) @ W_out + b_out (computed on host; tiny).
  - For np_i==1 rows only columns with np_j==1 & bert_j==0 survive, so we
    gather those ~R=1030 rows / ~M=535 kv columns on the host and run dense
    attention over the gathered set on device (~8x less work than dense).

Sharding: core = (batch b, head-pair g): 2 batches x 4 head groups.
  W_qkv split column-wise per head pair, W_out row-wise; each core produces
  a partial [R,512] output; host sums the 4 partials per batch.

All matmul operands are bf16 (PSUM accumulation stays fp32): 1 cyc/row on
the PE at any free size vs 4 for fp32, half DMA/SBUF/LDWEIGHTS cost. A host
simulation of 8-bit-mantissa rounding through the whole pipeline gives
~2.6e-3 scale-relative error vs the 2e-2 gate.

Device dataflow per core (R_PAD query rows, M_PAD kv cols, 2 heads):
  xT [512, R_PAD] ships pre-gathered/transposed (kv rows first, then tail
  rows); kvc [128, NMT] is the kv-indicator column per m-tile.
  1. K^T = Wk^T x^T [128, M_PAD]; Q^T chunks computed lazily per r-chunk.
  2. Per r-chunk (PSUM-bank-sized, <=512), per m-tile: V_aug built lazily on
     the first pass: V rows scaled by kvc, plus kvc replicated into 64
     stationary columns per head so the attn@V matmul emits the softmax
     denominator REPLICATED on psum partitions 0:64 for free.
     S^T pair (both heads, disjoint PE row groups) -> one exp (ScalarE,
     2-bank PSUM AP -> bf16 SBUF) -> O^T += V_aug_h^T P_h^T accumulated
     over m-tiles into a 2-bank pair.
  3. recip = 1/denoms on the 64 replicated partitions directly (DVE approx),
     O^T normalized by tensor_mul -> OnT bf16; y = OnT^T @ W_out rows per
     128-r-tile as soon as both heads are normalized; DMA out interleaved.
  Host adds bias, sums the 4 head-group partials, fills masked rows.
"""

import numpy as np

_CORES = 8
_DIM = 512
_DH = 64
_H = 8
_INNER = _H * _DH


def _ceil_to(x, m):
    return ((x + m - 1) // m) * m


def _chunks(total, step):
    out = []
    o = 0
    while o < total:
        out.append((o, min(step, total - o)))
        o += step
    return out


def _chunks_ge(total, step=512, minc=256):
    """Chunks of <= step, each >= minc (rebalancing the tail)."""
    out = _chunks(total, step)
    if len(out) >= 2 and out[-1][1] < minc:
        o_prev, w_prev = out[-2]
        o_last, w_last = out[-1]
        move = minc - w_last
        out[-2] = (o_prev, w_prev - move)
        out[-1] = (o_last - move, w_last + move)
    return out


def build_bass(R_PAD, M_PAD):
    """Build the SPMD bass program for padded sizes R_PAD (queries) and
    M_PAD (kv columns). Returns the compiled Bacc object."""
    import concourse.bacc as bacc
    import concourse.mybir as mybir
    import concourse.tile as tile

    f32 = mybir.dt.float32
    bf16 = mybir.dt.bfloat16
    EXP = mybir.ActivationFunctionType.Exp

    assert R_PAD % 16 == 0 and M_PAD % 128 == 0 and R_PAD >= M_PAD
    NMT = M_PAD // 128          # kv m-tiles
    NRT = (R_PAD + 127) // 128  # query r-tiles for the final projection
    RC = _chunks_ge(R_PAD)      # r-chunks: one PSUM bank each, >=256
    MC = _chunks_ge(M_PAD)
    assert len(RC) <= 3

    nc = bacc.Bacc("TRN2", target_bir_lowering=False, debug=False,
                   num_devices=_CORES)

    xT_d = nc.dram_tensor("xT", [512, R_PAD], bf16, kind="ExternalInput")
    wq_d = nc.dram_tensor("wq", [512, 128], bf16, kind="ExternalInput")
    wk_d = nc.dram_tensor("wk", [512, 128], bf16, kind="ExternalInput")
    wv_d = nc.dram_tensor("wv", [512, 128], bf16, kind="ExternalInput")
    kvc_d = nc.dram_tensor("kvc", [128, NMT], f32, kind="ExternalInput")
    wo_d = nc.dram_tensor("wo", [128, 512], bf16, kind="ExternalInput")
    y_d = nc.dram_tensor("y", [R_PAD, 512], f32, kind="ExternalOutput")

    with tile.TileContext(nc) as tc:
        with (
            tc.tile_pool(name="consts", bufs=1) as consts,
            tc.tile_pool(name="pt", bufs=3) as ptpool,
            tc.tile_pool(name="rcp", bufs=2) as rpool,
            tc.tile_pool(name="ysb", bufs=4) as ypool,
            tc.tile_pool(name="psA", bufs=2, space="PSUM") as psA,
            tc.tile_pool(name="psB", bufs=2, space="PSUM") as psB,
        ):
            # ---- input DMAs: first-needed first, issue split across
            # engines so issuance cost (~0.7us each) is parallel ----------
            wq = consts.tile([128, 4, 128], bf16, tag="wq")
            nc.sync.dma_start(
                out=wq, in_=wq_d.ap().rearrange("(a p) d -> p a d", p=128))
            wk = consts.tile([128, 4, 128], bf16, tag="wk")
            nc.scalar.dma_start(
                out=wk, in_=wk_d.ap().rearrange("(a p) d -> p a d", p=128))
            wv = consts.tile([128, 4, 128], bf16, tag="wv")
            nc.gpsimd.dma_start(
                out=wv, in_=wv_d.ap().rearrange("(a p) d -> p a d", p=128))
            xT = consts.tile([128, 4, R_PAD], bf16, tag="xT")
            xeng = [nc.sync, nc.scalar, nc.gpsimd, nc.sync]
            for c in range(4):
                xeng[c].dma_start(
                    out=xT[:, c, :], in_=xT_d.ap()[c * 128:(c + 1) * 128, :])
            kvc = consts.tile([128, NMT], f32, tag="kvc")
            nc.gpsimd.dma_start(out=kvc, in_=kvc_d.ap())
            wo = consts.tile([128, 512], bf16, tag="wo")
            nc.sync.dma_start(out=wo, in_=wo_d.ap())

            ones = consts.tile([128, 2, 64], bf16, tag="ones")
            nc.gpsimd.memset(ones, 1.0)

            # ---- K projection (Q is computed lazily per r-chunk below) --
            KT = consts.tile([128, M_PAD], bf16, tag="KT")
            for i, (o, w) in enumerate(MC):
                ps = psB.tile([128, 2, 512], f32, tag="B", name=f"kps{i}")
                for c in range(4):
                    nc.tensor.matmul(ps[:, 0, :w], wk[:, c, :],
                                     xT[:, c, o:o + w],
                                     start=(c == 0), stop=(c == 3))
                nc.scalar.copy(KT[:, o:o + w], ps[:, 0, :w])

            # ---- main loop: per r-chunk, per m-tile: S pair -> exp ->
            # O accumulate; V_aug built lazily on the first r-chunk --------
            QT = consts.tile([128, R_PAD], bf16, tag="QT")
            OnT = consts.tile([128, R_PAD], bf16, tag="OnT")
            # y r-tile groups: consecutive FULL tiles pair up; a partial
            # trailing tile is emitted alone.
            n_full = R_PAD // 128
            YG = [(i, i + 1) for i in range(0, n_full - 1, 2)]
            if n_full % 2 == 1:
                YG.append((n_full - 1,))
            if R_PAD % 128 != 0:
                YG.append((n_full,))
            V = []
            ydone = 0
            for ci, (o, w) in enumerate(RC):
                # lazy Q projection for this r-chunk
                qps = psA.tile([128, 2, 512], f32, tag="A", name=f"qps{ci}")
                for c in range(4):
                    nc.tensor.matmul(qps[:, 0, :w], wq[:, c, :],
                                     xT[:, c, o:o + w],
                                     start=(c == 0), stop=(c == 3))
                if ci == 0:
                    nc.scalar.copy(QT[:, o:o + w], qps[:, 0, :w])
                else:
                    nc.vector.tensor_copy(QT[:, o:o + w], qps[:, 0, :w])

                op = psB.tile([128, 2, 512], f32, tag="B", name=f"op{ci}")
                for mt in range(NMT):
                    msl = slice(mt * 128, (mt + 1) * 128)
                    if ci == 0:
                        # lazy V_aug: [kvc x64 | V_h x64] per head; the kvc
                        # columns make the O matmul emit the softmax
                        # denominator replicated on partitions 0:64, and
                        # null the tail rows sitting below M_PAD. The V
                        # matmul is split per head so each head's 64 V
                        # columns land in its own PSUM bank, letting one
                        # DVE op scale-copy both heads at once.
                        vps = psA.tile([128, 2, 512], f32, tag="A",
                                       name=f"vps{mt}")
                        for c in range(4):
                            for h in range(2):
                                nc.tensor.matmul(
                                    vps[:, h, 0:64], xT[:, c, msl],
                                    wv[:, c, h * 64:(h + 1) * 64],
                                    start=(c == 0), stop=(c == 3))
                        vt = consts.tile([128, 2, 128], bf16, tag=f"v{mt}",
                                         name=f"v{mt}")
                        nc.gpsimd.tensor_scalar_mul(
                            vt[:, :, 0:64], in0=ones,
                            scalar1=kvc[:, mt:mt + 1])
                        nc.vector.tensor_scalar_mul(
                            vt[:, :, 64:128], in0=vps[:, :, 0:64],
                            scalar1=kvc[:, mt:mt + 1])
                        V.append(vt)
                    sp = psA.tile([128, 2, 512], f32, tag="A",
                                  name=f"sp{ci}_{mt}")
                    for h in range(2):
                        hs = slice(h * 64, (h + 1) * 64)
                        nc.tensor.matmul(sp[:, h, :w], KT[hs, msl],
                                         QT[hs, o:o + w],
                                         start=True, stop=True)
                    pt = ptpool.tile([128, 2, 512], bf16, tag="pt",
                                     name=f"pt{ci}_{mt}")
                    nc.scalar.activation(out=pt[:, :, :w], in_=sp[:, :, :w],
                                         func=EXP)
                    for h in range(2):
                        nc.tensor.matmul(op[:, h, :w], V[mt][:, h, :],
                                         pt[:, h, :w],
                                         start=(mt == 0), stop=(mt == NMT - 1))

                # normalize: denominators sit replicated on partitions 0:64
                rc = rpool.tile([64, 2, 512], f32, tag="rcp", name=f"rc{ci}")
                nc.vector.reciprocal_approx_fast(rc[:, :, :w],
                                                 op[0:64, :, :w])
                for h in range(2):
                    nc.vector.tensor_mul(OnT[h * 64:(h + 1) * 64, o:o + w],
                                         op[64:128, h, :w], rc[:, h, :w])

                # out projection for r-tile groups fully covered so far;
                # two full r-tiles share one 2-bank PSUM tile so a single
                # copy + DMA moves both (copies alternate DVE / ScalarE).
                done = o + w
                while ydone < len(YG):
                    rts = YG[ydone]
                    lo = rts[0] * 128
                    hi = min(rts[-1] * 128 + 128, R_PAD)
                    if hi > done:
                        break
                    yp = psA.tile([128, 2, 512], f32, tag="A",
                                  name=f"yp{rts[0]}")
                    for j, rt in enumerate(rts):
                        tw = min(128, R_PAD - rt * 128)
                        nc.tensor.matmul(
                            yp[:tw, j, :], OnT[:, rt * 128:rt * 128 + tw],
                            wo, start=True, stop=True)
                    copy = (nc.vector.tensor_copy if ydone % 2 == 0
                            else nc.scalar.copy)
                    ysb = ypool.tile([128, 2, 512], f32, tag="y")
                    if len(rts) == 2:
                        copy(ysb, yp)
                        nc.sync.dma_start(
                            out=y_d.ap()[lo:hi, :].rearrange(
                                "(a p) d -> p a d", p=128),
                            in_=ysb)
                    else:
                        copy(ysb[:hi - lo, 0, :], yp[:hi - lo, 0, :])
                        nc.sync.dma_start(out=y_d.ap()[lo:hi, :],
                                          in_=ysb[:hi - lo, 0, :])
                    ydone += 1

    nc.compile()
    return nc


def _prep(x, mask_np, mask_bert, W_qkv, W_out):
    """Host-side gather/shard. Returns (in_maps, meta)."""
    import ml_dtypes
    bf16 = ml_dtypes.bfloat16

    B, N, DIM = x.shape
    assert (B, DIM) == (2, _DIM)
    x = np.ascontiguousarray(x, dtype=np.float32)
    W_qkv = np.ascontiguousarray(W_qkv, dtype=np.float32)
    W_out = np.ascontiguousarray(W_out, dtype=np.float32)

    kv_idx, tail_idx, Ms, tails = [], [], [], []
    for b in range(B):
        npb = mask_np[b].astype(bool)
        bb = mask_bert[b].astype(bool)
        kv = np.nonzero(npb & ~bb)[0]
        tl = np.nonzero(npb & bb)[0]
        kv_idx.append(kv)
        tail_idx.append(tl)
        Ms.append(len(kv))
        tails.append(len(tl))

    M_PAD = max(128, _ceil_to(max(Ms), 128))
    # rows are packed [kv | tail] with no gap: the tail rows that fall in
    # [M_b, M_PAD) act as key/value candidates but are nulled by the kvc
    # indicator (V rows scaled to 0, denominator columns 0).
    R_PAD = max(128, _ceil_to(max(Ms[b] + tails[b] for b in range(B)), 16),
                M_PAD)

    NMT = M_PAD // 128
    xT_b, kvc_b, row_pos = [], [], []
    for b in range(B):
        xa = np.zeros((512, R_PAD), dtype=np.float32)
        xa[:, :Ms[b]] = x[b][kv_idx[b]].T
        xa[:, Ms[b]:Ms[b] + tails[b]] = x[b][tail_idx[b]].T
        xT_b.append(np.ascontiguousarray(xa.astype(bf16)))
        kvones = np.zeros(M_PAD, dtype=np.float32)
        kvones[:Ms[b]] = 1.0
        kvc_b.append(np.ascontiguousarray(kvones.reshape(NMT, 128).T))
        # output row p of the device result corresponds to token row_pos[p]
        pos = np.concatenate([kv_idx[b], tail_idx[b]])
        row_pos.append(pos)

    scale = np.float32(_DH ** -0.5)
    in_maps = []
    for c in range(_CORES):
        b, g = divmod(c, 4)
        qc = slice(128 * g, 128 * g + 128)
        kc = slice(_INNER + 128 * g, _INNER + 128 * g + 128)
        vc = slice(2 * _INNER + 128 * g, 2 * _INNER + 128 * g + 128)
        wq = np.ascontiguousarray((W_qkv[:, qc] * scale).astype(bf16))
        wk = np.ascontiguousarray(W_qkv[:, kc].astype(bf16))
        wv = np.ascontiguousarray(W_qkv[:, vc].astype(bf16))
        wo = np.ascontiguousarray(
            W_out[128 * g:128 * g + 128, :].astype(bf16))
        in_maps.append({"xT": xT_b[b], "wq": wq, "wk": wk, "wv": wv, "wo": wo,
                        "kvc": kvc_b[b]})

    meta = dict(M_PAD=M_PAD, R_PAD=R_PAD, Ms=Ms, tails=tails,
                kv_idx=kv_idx, tail_idx=tail_idx, row_pos=row_pos)
    return in_maps, meta


def _assemble(results, meta, x, mask_np, W_qkv, W_out, b_out):
    B, N, _ = x.shape
    out = np.empty((B, N, _DIM), dtype=np.float32)
    Wv_full = W_qkv[:, 2 * _INNER:].astype(np.float32)
    for b in range(B):
        # constant output for fully-masked rows: uniform attention = mean(V)
        meanv = (x[b].mean(axis=0, dtype=np.float32) @ Wv_full)
        yconst = meanv @ W_out.astype(np.float32) + b_out
        out[b, :, :] = yconst[None, :]
        Mb, tb = meta["Ms"][b], meta["tails"][b]
        if Mb == 0:
            # no unmasked kv columns: every row fully masked -> uniform
            continue
        acc = None
        for g in range(4):
            yp = results[4 * b + g]["y"]
            acc = yp.copy() if acc is None else acc + yp
        out[b, meta["row_pos"][b], :] = acc[:Mb + tb] + b_out
    return out


_CACHE = {}


def _get_bass(R_PAD, M_PAD):
    key = (R_PAD, M_PAD)
    if key not in _CACHE:
        _CACHE[key] = build_bass(R_PAD, M_PAD)
    return _CACHE[key]


def run_spmd(in_maps, meta, trace=False, tmpdir=None, trace_cores=None):
    from concourse.bass_utils import run_bass_kernel_spmd

    nc = _get_bass(meta["R_PAD"], meta["M_PAD"])
    return run_bass_kernel_spmd(
        nc, in_maps, core_ids=list(range(_CORES)), trace=trace, tmpdir=tmpdir,
        trace_cores=trace_cores)


def kernel(x, mask_np, mask_bert, W_qkv, W_out, b_out):
    x = np.asarray(x)
    mask_np = np.asarray(mask_np)
    mask_bert = np.asarray(mask_bert)
    W_qkv = np.asarray(W_qkv, dtype=np.float32)
    W_out = np.asarray(W_out, dtype=np.float32)
    b_out = np.asarray(b_out, dtype=np.float32)

    in_maps, meta = _prep(x, mask_np, mask_bert, W_qkv, W_out)
    res = run_spmd(in_maps, meta)
    return _assemble(res.results, meta, x, mask_np, W_qkv, W_out, b_out)


# revision 20
# speedup vs baseline: 1.6571x; 1.0580x over previous
"""Sparse dual-masked attention for Trainium2, 8 NeuronCores.

Problem: B=2, N=2048, DIM=512, H=8, DH=64.
  qkv = x @ W_qkv; per-head attention with dual mask
  (np_i*np_j==0 | bert_j==1 -> -1000), softmax, out proj + bias.

Structure exploited (sparse_attention):
  - A row i with np_i==0 is fully masked -> softmax uniform -> output row is
    the constant mean(V) @ W_out + b_out (computed on host; tiny).
  - For np_i==1 rows only columns with np_j==1 & bert_j==0 survive, so we
    gather those ~R=1030 rows / ~M=535 kv columns on the host and run dense
    attention over the gathered set on device (~8x less work than dense).

Sharding: core = (batch b, head-pair g): 2 batches x 4 head groups.
  W_qkv split column-wise per head pair, W_out row-wise; each core produces
  a partial [R,512] output; host sums the 4 partials per batch.

All matmul operands are bf16 (PSUM accumulation stays fp32): 1 cyc/row on
the PE at any free size vs 4 for fp32, half DMA/SBUF/LDWEIGHTS cost. A host
simulation of 8-bit-mantissa rounding through the whole pipeline gives
~2.6e-3 scale-relative error vs the 2e-2 gate.

Device dataflow per core (R_PAD query rows, M_PAD kv cols, 2 heads):
  xT [512, R_PAD] ships pre-gathered/transposed (kv rows first, then tail
  rows); kvc [128, NMT] is the kv-indicator column per m-tile.
  1. K^T = Wk^T x^T [128, M_PAD]; Q^T chunks computed lazily per r-chunk.
  2. Per r-chunk (PSUM-bank-sized, <=512), per m-tile: V_aug built lazily on
     the first pass: V rows scaled by kvc, plus kvc replicated into 64
     stationary columns per head so the attn@V matmul emits the softmax
     denominator REPLICATED on psum partitions 0:64 for free.
     S^T pair (both heads, disjoint PE row groups) -> one exp (ScalarE,
     2-bank PSUM AP -> bf16 SBUF) -> O^T += V_aug_h^T P_h^T accumulated
     over m-tiles into a 2-bank pair.
  3. recip = 1/denoms on the 64 replicated partitions directly (DVE approx),
     O^T normalized by tensor_mul -> OnT bf16; y = OnT^T @ W_out rows per
     128-r-tile as soon as both heads are normalized; DMA out interleaved.
  Host adds bias, sums the 4 head-group partials, fills masked rows.
"""

import numpy as np

_CORES = 8
_DIM = 512
_DH = 64
_H = 8
_INNER = _H * _DH


def _ceil_to(x, m):
    return ((x + m - 1) // m) * m


def _chunks(total, step):
    out = []
    o = 0
    while o < total:
        out.append((o, min(step, total - o)))
        o += step
    return out


def _chunks_ge(total, step=512, minc=256):
    """Chunks of <= step, each >= minc (rebalancing the tail)."""
    out = _chunks(total, step)
    if len(out) >= 2 and out[-1][1] < minc:
        o_prev, w_prev = out[-2]
        o_last, w_last = out[-1]
        move = minc - w_last
        out[-2] = (o_prev, w_prev - move)
        out[-1] = (o_last - move, w_last + move)
    return out


def build_bass(R_PAD, M_PAD):
    """Build the SPMD bass program for padded sizes R_PAD (queries) and
    M_PAD (kv columns). Returns the compiled Bacc object."""
    import concourse.bacc as bacc
    import concourse.mybir as mybir
    import concourse.tile as tile

    f32 = mybir.dt.float32
    bf16 = mybir.dt.bfloat16
    EXP = mybir.ActivationFunctionType.Exp

    assert R_PAD % 16 == 0 and M_PAD % 128 == 0 and R_PAD >= M_PAD
    NMT = M_PAD // 128          # kv m-tiles
    NRT = (R_PAD + 127) // 128  # query r-tiles for the final projection
    RC = _chunks_ge(R_PAD)      # r-chunks: one PSUM bank each, >=256
    MC = _chunks_ge(M_PAD)
    assert len(RC) <= 3

    nc = bacc.Bacc("TRN2", target_bir_lowering=False, debug=False,
                   num_devices=_CORES)

    xT_d = nc.dram_tensor("xT", [512, R_PAD], bf16, kind="ExternalInput")
    wq_d = nc.dram_tensor("wq", [512, 128], bf16, kind="ExternalInput")
    wk_d = nc.dram_tensor("wk", [512, 128], bf16, kind="ExternalInput")
    wv_d = nc.dram_tensor("wv", [512, 128], bf16, kind="ExternalInput")
    kvc_d = nc.dram_tensor("kvc", [128, NMT], f32, kind="ExternalInput")
    kvr_d = nc.dram_tensor("kvr", [128, NMT * 128], bf16,
                           kind="ExternalInput")
    wo_d = nc.dram_tensor("wo", [128, 512], bf16, kind="ExternalInput")
    y_d = nc.dram_tensor("y", [R_PAD, 512], bf16, kind="ExternalOutput")

    with tile.TileContext(nc) as tc:
        with (
            tc.tile_pool(name="consts", bufs=1) as consts,
            tc.tile_pool(name="pt", bufs=4) as ptpool,
            tc.tile_pool(name="rcp", bufs=2) as rpool,
            tc.tile_pool(name="ysb", bufs=4) as ypool,
            tc.tile_pool(name="psA", bufs=2, space="PSUM") as psA,
            tc.tile_pool(name="psB", bufs=2, space="PSUM") as psB,
        ):
            # ---- input DMAs: first-needed first; xT split into the kv
            # column prefix [0:M_PAD] (needed by K/V and the first S) and
            # the tail, spread over the 3 DMA-capable queues -------------
            wq = consts.tile([128, 4, 128], bf16, tag="wq")
            nc.sync.dma_start(
                out=wq, in_=wq_d.ap().rearrange("(a p) d -> p a d", p=128))
            wk = consts.tile([128, 4, 128], bf16, tag="wk")
            nc.scalar.dma_start(
                out=wk, in_=wk_d.ap().rearrange("(a p) d -> p a d", p=128))
            wv = consts.tile([128, 4, 128], bf16, tag="wv")
            nc.gpsimd.dma_start(
                out=wv, in_=wv_d.ap().rearrange("(a p) d -> p a d", p=128))
            xT = consts.tile([128, 4, R_PAD], bf16, tag="xT")
            Vt = consts.tile([128, NMT, 2, 128], bf16, tag="Vt")
            xeng = [nc.sync, nc.scalar, nc.gpsimd, nc.sync]
            for c in range(4):
                xeng[c].dma_start(
                    out=xT[:, c, 0:M_PAD],
                    in_=xT_d.ap()[c * 128:(c + 1) * 128, 0:M_PAD])
            kvc = consts.tile([128, NMT], f32, tag="kvc")
            nc.gpsimd.dma_start(out=kvc, in_=kvc_d.ap())
            nc.gpsimd.dma_start(
                out=Vt[:, :, :, 0:64],
                in_=kvr_d.ap().rearrange("p (a b c) -> p a b c", b=2, c=64))
            for c in range(4):
                xeng[c].dma_start(
                    out=xT[:, c, M_PAD:R_PAD],
                    in_=xT_d.ap()[c * 128:(c + 1) * 128, M_PAD:R_PAD])
            wo = consts.tile([128, 512], bf16, tag="wo")
            nc.scalar.dma_start(out=wo, in_=wo_d.ap())

            # ---- K projection (Q is computed lazily per r-chunk below) --
            KT = consts.tile([128, M_PAD], bf16, tag="KT")
            for i, (o, w) in enumerate(MC):
                ps = psB.tile([128, 2, 512], f32, tag="B", name=f"kps{i}")
                for c in range(4):
                    nc.tensor.matmul(ps[:, 0, :w], wk[:, c, :],
                                     xT[:, c, o:o + w],
                                     start=(c == 0), stop=(c == 3))
                nc.scalar.copy(KT[:, o:o + w], ps[:, 0, :w])

            # ---- main loop: per r-chunk, per m-tile: S pair -> exp ->
            # O accumulate; V_aug built lazily on the first r-chunk --------
            QT = consts.tile([128, R_PAD], bf16, tag="QT")
            OnT = consts.tile([128, R_PAD], bf16, tag="OnT")
            # y r-tile groups: consecutive FULL tiles pair up; a partial
            # trailing tile is emitted alone.
            n_full = R_PAD // 128
            YG = [(i, i + 1) for i in range(0, n_full - 1, 2)]
            if n_full % 2 == 1:
                YG.append((n_full - 1,))
            if R_PAD % 128 != 0:
                YG.append((n_full,))
            ydone = 0
            for ci, (o, w) in enumerate(RC):
                # lazy Q projection for this r-chunk
                qps = psA.tile([128, 2, 512], f32, tag="A", name=f"qps{ci}")
                for c in range(4):
                    nc.tensor.matmul(qps[:, 0, :w], wq[:, c, :],
                                     xT[:, c, o:o + w],
                                     start=(c == 0), stop=(c == 3))
                if ci == 0:
                    nc.scalar.copy(QT[:, o:o + w], qps[:, 0, :w])
                else:
                    nc.vector.tensor_copy(QT[:, o:o + w], qps[:, 0, :w])

                op = psB.tile([128, 2, 512], f32, tag="B", name=f"op{ci}")
                pts = {}
                for mt in range(NMT):
                    msl = slice(mt * 128, (mt + 1) * 128)
                    if ci == 0:
                        # lazy V_aug into Vt[:, mt]: [kvc x64 | V_h x64] per
                        # head; the kvc columns (DMA'd from host) make the O
                        # matmul emit the softmax denominator replicated on
                        # partitions 0:64 and null the tail rows sitting
                        # below M_PAD. The V matmul is split per head so
                        # each head's V columns land in separate PSUM banks,
                        # letting one DVE op scale-copy both heads at once.
                        vps = psA.tile([128, 2, 512], f32, tag="A",
                                       name=f"vps{mt}")
                        for c in range(4):
                            for h in range(2):
                                nc.tensor.matmul(
                                    vps[:, h, 0:64], xT[:, c, msl],
                                    wv[:, c, h * 64:(h + 1) * 64],
                                    start=(c == 0), stop=(c == 3))
                        nc.vector.tensor_scalar_mul(
                            Vt[:, mt, :, 64:128], in0=vps[:, :, 0:64],
                            scalar1=kvc[:, mt:mt + 1])
                    sp = psA.tile([128, 2, 512], f32, tag="A",
                                  name=f"sp{ci}_{mt}")
                    for h in range(2):
                        hs = slice(h * 64, (h + 1) * 64)
                        nc.tensor.matmul(sp[:, h, :w], KT[hs, msl],
                                         QT[hs, o:o + w],
                                         start=True, stop=True)
                    pt = ptpool.tile([128, 2, 512], bf16, tag="pt",
                                     name=f"pt{ci}_{mt}")
                    nc.scalar.activation(out=pt[:, :, :w], in_=sp[:, :, :w],
                                         func=EXP)
                    pts[mt] = pt
                    # O for the PREVIOUS m-tile: keeps the PE busy on
                    # S(mt) while ScalarE runs exp(mt-1) instead of the
                    # in-order PE queue stalling on exp(mt).
                    if mt > 0:
                        for h in range(2):
                            nc.tensor.matmul(op[:, h, :w],
                                             Vt[:, mt - 1, h, :],
                                             pts[mt - 1][:, h, :w],
                                             start=(mt == 1), stop=False)
                for h in range(2):
                    nc.tensor.matmul(op[:, h, :w], Vt[:, NMT - 1, h, :],
                                     pts[NMT - 1][:, h, :w],
                                     start=False, stop=True)

                # normalize: denominators sit replicated on partitions 0:64
                rc = rpool.tile([64, 2, 512], f32, tag="rcp", name=f"rc{ci}")
                nc.vector.reciprocal_approx_fast(rc[:, :, :w],
                                                 op[0:64, :, :w])
                for h in range(2):
                    nc.vector.tensor_mul(OnT[h * 64:(h + 1) * 64, o:o + w],
                                         op[64:128, h, :w], rc[:, h, :w])

                # out projection for r-tile groups fully covered so far;
                # two full r-tiles share one 2-bank PSUM tile so a single
                # copy + DMA moves both (copies alternate DVE / ScalarE).
                done = o + w
                while ydone < len(YG):
                    rts = YG[ydone]
                    lo = rts[0] * 128
                    hi = min(rts[-1] * 128 + 128, R_PAD)
                    if hi > done:
                        break
                    yp = psA.tile([128, 2, 512], f32, tag="A",
                                  name=f"yp{rts[0]}")
                    for j, rt in enumerate(rts):
                        tw = min(128, R_PAD - rt * 128)
                        nc.tensor.matmul(
                            yp[:tw, j, :], OnT[:, rt * 128:rt * 128 + tw],
                            wo, start=True, stop=True)
                    copy = (nc.vector.tensor_copy if ydone % 2 == 0
                            else nc.scalar.copy)
                    deng = [nc.sync, nc.scalar, nc.gpsimd][ydone % 3]
                    ysb = ypool.tile([128, 2, 512], bf16, tag="y")
                    if len(rts) == 2:
                        copy(ysb, yp)
                        deng.dma_start(
                            out=y_d.ap()[lo:hi, :].rearrange(
                                "(a p) d -> p a d", p=128),
                            in_=ysb)
                    else:
                        copy(ysb[:hi - lo, 0, :], yp[:hi - lo, 0, :])
                        deng.dma_start(out=y_d.ap()[lo:hi, :],
                                       in_=ysb[:hi - lo, 0, :])
                    ydone += 1

    nc.compile()
    return nc


def _prep(x, mask_np, mask_bert, W_qkv, W_out):
    """Host-side gather/shard. Returns (in_maps, meta)."""
    import ml_dtypes
    bf16 = ml_dtypes.bfloat16

    B, N, DIM = x.shape
    assert (B, DIM) == (2, _DIM)
    x = np.ascontiguousarray(x, dtype=np.float32)
    W_qkv = np.ascontiguousarray(W_qkv, dtype=np.float32)
    W_out = np.ascontiguousarray(W_out, dtype=np.float32)

    kv_idx, tail_idx, Ms, tails = [], [], [], []
    for b in range(B):
        npb = mask_np[b].astype(bool)
        bb = mask_bert[b].astype(bool)
        kv = np.nonzero(npb & ~bb)[0]
        tl = np.nonzero(npb & bb)[0]
        kv_idx.append(kv)
        tail_idx.append(tl)
        Ms.append(len(kv))
        tails.append(len(tl))

    M_PAD = max(128, _ceil_to(max(Ms), 128))
    # rows are packed [kv | tail] with no gap: the tail rows that fall in
    # [M_b, M_PAD) act as key/value candidates but are nulled by the kvc
    # indicator (V rows scaled to 0, denominator columns 0).
    R_PAD = max(128, _ceil_to(max(Ms[b] + tails[b] for b in range(B)), 16),
                M_PAD)

    NMT = M_PAD // 128
    xT_b, kvc_b, kvr_b, row_pos = [], [], [], []
    for b in range(B):
        xa = np.zeros((512, R_PAD), dtype=np.float32)
        xa[:, :Ms[b]] = x[b][kv_idx[b]].T
        xa[:, Ms[b]:Ms[b] + tails[b]] = x[b][tail_idx[b]].T
        xT_b.append(np.ascontiguousarray(xa.astype(bf16)))
        kvones = np.zeros(M_PAD, dtype=np.float32)
        kvones[:Ms[b]] = 1.0
        kvc_b.append(np.ascontiguousarray(kvones.reshape(NMT, 128).T))
        # kvc replicated into the 64 denominator columns per (m-tile, head)
        kvr = np.broadcast_to(kvc_b[b][:, :, None, None],
                              (128, NMT, 2, 64))
        kvr_b.append(np.ascontiguousarray(
            kvr.reshape(128, NMT * 128).astype(bf16)))
        # output row p of the device result corresponds to token row_pos[p]
        pos = np.concatenate([kv_idx[b], tail_idx[b]])
        row_pos.append(pos)

    scale = np.float32(_DH ** -0.5)
    in_maps = []
    for c in range(_CORES):
        b, g = divmod(c, 4)
        qc = slice(128 * g, 128 * g + 128)
        kc = slice(_INNER + 128 * g, _INNER + 128 * g + 128)
        vc = slice(2 * _INNER + 128 * g, 2 * _INNER + 128 * g + 128)
        wq = np.ascontiguousarray((W_qkv[:, qc] * scale).astype(bf16))
        wk = np.ascontiguousarray(W_qkv[:, kc].astype(bf16))
        wv = np.ascontiguousarray(W_qkv[:, vc].astype(bf16))
        wo = np.ascontiguousarray(
            W_out[128 * g:128 * g + 128, :].astype(bf16))
        in_maps.append({"xT": xT_b[b], "wq": wq, "wk": wk, "wv": wv, "wo": wo,
                        "kvc": kvc_b[b], "kvr": kvr_b[b]})

    meta = dict(M_PAD=M_PAD, R_PAD=R_PAD, Ms=Ms, tails=tails,
                kv_idx=kv_idx, tail_idx=tail_idx, row_pos=row_pos)
    return in_maps, meta


def _assemble(results, meta, x, mask_np, W_qkv, W_out, b_out):
    B, N, _ = x.shape
    out = np.empty((B, N, _DIM), dtype=np.float32)
    Wv_full = W_qkv[:, 2 * _INNER:].astype(np.float32)
    for b in range(B):
        # constant output for fully-masked rows: uniform attention = mean(V)
        meanv = (x[b].mean(axis=0, dtype=np.float32) @ Wv_full)
        yconst = meanv @ W_out.astype(np.float32) + b_out
        out[b, :, :] = yconst[None, :]
        Mb, tb = meta["Ms"][b], meta["tails"][b]
        if Mb == 0:
            # no unmasked kv columns: every row fully masked -> uniform
            continue
        acc = None
        for g in range(4):
            yp = results[4 * b + g]["y"].astype(np.float32)
            acc = yp if acc is None else acc + yp
        out[b, meta["row_pos"][b], :] = acc[:Mb + tb] + b_out
    return out


_CACHE = {}


def _get_bass(R_PAD, M_PAD):
    key = (R_PAD, M_PAD)
    if key not in _CACHE:
        _CACHE[key] = build_bass(R_PAD, M_PAD)
    return _CACHE[key]


def run_spmd(in_maps, meta, trace=False, tmpdir=None, trace_cores=None):
    from concourse.bass_utils import run_bass_kernel_spmd

    nc = _get_bass(meta["R_PAD"], meta["M_PAD"])
    return run_bass_kernel_spmd(
        nc, in_maps, core_ids=list(range(_CORES)), trace=trace, tmpdir=tmpdir,
        trace_cores=trace_cores)


def kernel(x, mask_np, mask_bert, W_qkv, W_out, b_out):
    x = np.asarray(x)
    mask_np = np.asarray(mask_np)
    mask_bert = np.asarray(mask_bert)
    W_qkv = np.asarray(W_qkv, dtype=np.float32)
    W_out = np.asarray(W_out, dtype=np.float32)
    b_out = np.asarray(b_out, dtype=np.float32)

    in_maps, meta = _prep(x, mask_np, mask_bert, W_qkv, W_out)
    res = run_spmd(in_maps, meta)
    return _assemble(res.results, meta, x, mask_np, W_qkv, W_out, b_out)


# revision 27
# speedup vs baseline: 1.9549x; 1.1797x over previous
"""Sparse dual-masked attention for Trainium2, 8 NeuronCores.

Problem: B=2, N=2048, DIM=512, H=8, DH=64.
  qkv = x @ W_qkv; per-head attention with dual mask
  (np_i*np_j==0 | bert_j==1 -> -1000), softmax, out proj + bias.

Structure exploited (sparse_attention):
  - A row i with np_i==0 is fully masked -> softmax uniform -> output row is
    the constant mean(V) @ W_out + b_out (computed on host; tiny).
  - For np_i==1 rows only columns with np_j==1 & bert_j==0 survive, so we
    gather those ~R=1030 rows / ~M=535 kv columns on the host and run dense
    attention over the gathered set on device (~8x less work than dense).

Sharding: core = (batch b, head-pair g): 2 batches x 4 head groups.
  W_qkv split column-wise per head pair, W_out row-wise; each core produces
  a partial [R,512] output; host sums the 4 partials per batch.

All matmul operands are bf16 (PSUM accumulation stays fp32): 1 cyc/row on
the PE at any free size vs 4 for fp32, half DMA/SBUF/LDWEIGHTS cost. A host
simulation of 8-bit-mantissa rounding through the whole pipeline gives
~2.6e-3 scale-relative error vs the 2e-2 gate.

Device dataflow per core (R_PAD query rows, M_PAD kv cols, 2 heads):
  xT [512, R_PAD] ships pre-gathered/transposed (kv rows first, then tail
  rows); kvc [128, NMT] is the kv-indicator column per m-tile.
  1. K^T = Wk^T x^T [128, M_PAD]; Q^T chunks computed lazily per r-chunk.
  2. Per r-chunk (PSUM-bank-sized, <=512), per m-tile: V_aug built lazily on
     the first pass: V rows scaled by kvc, plus kvc replicated into 64
     stationary columns per head so the attn@V matmul emits the softmax
     denominator REPLICATED on psum partitions 0:64 for free.
     S^T pair (both heads, disjoint PE row groups) -> one exp (ScalarE,
     2-bank PSUM AP -> bf16 SBUF) -> O^T += V_aug_h^T P_h^T accumulated
     over m-tiles into a 2-bank pair.
  3. recip = 1/denoms on the 64 replicated partitions directly (DVE approx),
     O^T normalized by tensor_mul -> OnT bf16; y = OnT^T @ W_out rows per
     128-r-tile as soon as both heads are normalized; DMA out interleaved.
  Host adds bias, sums the 4 head-group partials, fills masked rows.
"""

import numpy as np

_CORES = 8
_DIM = 512
_DH = 64
_H = 8
_INNER = _H * _DH


def _ceil_to(x, m):
    return ((x + m - 1) // m) * m


def _chunks(total, step):
    out = []
    o = 0
    while o < total:
        out.append((o, min(step, total - o)))
        o += step
    return out


def _chunks_ge(total, step=512, minc=256):
    """Chunks of <= step, each >= minc (rebalancing the tail)."""
    out = _chunks(total, step)
    if len(out) >= 2 and out[-1][1] < minc:
        o_prev, w_prev = out[-2]
        o_last, w_last = out[-1]
        move = minc - w_last
        out[-2] = (o_prev, w_prev - move)
        out[-1] = (o_last - move, w_last + move)
    return out


def build_bass(R_PAD, M_PAD):
    """Build the SPMD bass program for padded sizes R_PAD (queries) and
    M_PAD (kv columns). Returns the compiled Bacc object."""
    import concourse.bacc as bacc
    import concourse.mybir as mybir
    import concourse.tile as tile

    f32 = mybir.dt.float32
    bf16 = mybir.dt.bfloat16
    EXP = mybir.ActivationFunctionType.Exp

    assert R_PAD % 16 == 0 and M_PAD % 128 == 0 and R_PAD >= M_PAD
    NMT = M_PAD // 128          # kv m-tiles
    NRT = (R_PAD + 127) // 128  # query r-tiles for the final projection
    RC = _chunks_ge(R_PAD)      # r-chunks: one PSUM bank each, >=256
    MC = _chunks_ge(M_PAD)
    assert len(RC) <= 3

    nc = bacc.Bacc("TRN2", target_bir_lowering=False, debug=False,
                   num_devices=_CORES)

    xT_d = nc.dram_tensor("xT", [512, R_PAD], bf16, kind="ExternalInput")
    wq_d = nc.dram_tensor("wq", [512, 128], bf16, kind="ExternalInput")
    wk_d = nc.dram_tensor("wk", [512, 128], bf16, kind="ExternalInput")
    wv_d = nc.dram_tensor("wv", [512, 128], bf16, kind="ExternalInput")
    kvc_d = nc.dram_tensor("kvc", [128, NMT], f32, kind="ExternalInput")
    kvr_d = nc.dram_tensor("kvr", [128, NMT * 128], bf16,
                           kind="ExternalInput")
    wo_d = nc.dram_tensor("wo", [128, 512], bf16, kind="ExternalInput")
    y_d = nc.dram_tensor("y", [R_PAD, 512], bf16, kind="ExternalOutput")

    with tile.TileContext(nc) as tc:
        with (
            tc.tile_pool(name="consts", bufs=1) as consts,
            tc.tile_pool(name="pt", bufs=4) as ptpool,
            tc.tile_pool(name="rcp", bufs=2) as rpool,
            tc.tile_pool(name="ysb", bufs=4) as ypool,
            tc.tile_pool(name="psS", bufs=2, space="PSUM") as psS,
            tc.tile_pool(name="psO", bufs=1, space="PSUM") as psO,
            tc.tile_pool(name="psQ", bufs=1, space="PSUM") as psQ,
        ):
            # ---- input DMAs: first-needed first; xT split into the kv
            # column prefix [0:M_PAD] (needed by K/V and the first S) and
            # the tail, spread over the 3 DMA-capable queues -------------
            wq = consts.tile([128, 4, 128], bf16, tag="wq")
            nc.sync.dma_start(
                out=wq, in_=wq_d.ap().rearrange("(a p) d -> p a d", p=128))
            wk = consts.tile([128, 4, 128], bf16, tag="wk")
            nc.scalar.dma_start(
                out=wk, in_=wk_d.ap().rearrange("(a p) d -> p a d", p=128))
            wv = consts.tile([128, 4, 128], bf16, tag="wv")
            nc.gpsimd.dma_start(
                out=wv, in_=wv_d.ap().rearrange("(a p) d -> p a d", p=128))
            xT = consts.tile([128, 4, R_PAD], bf16, tag="xT")
            Vt = consts.tile([128, NMT, 2, 128], bf16, tag="Vt")
            xeng = [nc.sync, nc.scalar, nc.gpsimd, nc.sync]
            for c in range(4):
                xeng[c].dma_start(
                    out=xT[:, c, 0:M_PAD],
                    in_=xT_d.ap()[c * 128:(c + 1) * 128, 0:M_PAD])
            kvc = consts.tile([128, NMT], f32, tag="kvc")
            nc.gpsimd.dma_start(out=kvc, in_=kvc_d.ap())
            nc.gpsimd.dma_start(
                out=Vt[:, :, :, 0:64],
                in_=kvr_d.ap().rearrange("p (a b c) -> p a b c", b=2, c=64))
            for c in range(4):
                xeng[c].dma_start(
                    out=xT[:, c, M_PAD:R_PAD],
                    in_=xT_d.ap()[c * 128:(c + 1) * 128, M_PAD:R_PAD])
            wo = consts.tile([128, 512], bf16, tag="wo")
            nc.scalar.dma_start(out=wo, in_=wo_d.ap())

            # ---- K projection (Q is computed lazily per r-chunk below) --
            KT = consts.tile([128, M_PAD], bf16, tag="KT")
            for i, (o, w) in enumerate(MC):
                ps = psO.tile([128, 2, 512], f32, tag="O", name=f"kps{i}")
                for c in range(4):
                    nc.tensor.matmul(ps[:, 0, :w], wk[:, c, :],
                                     xT[:, c, o:o + w],
                                     start=(c == 0), stop=(c == 3))
                nc.scalar.copy(KT[:, o:o + w], ps[:, 0, :w])

            # ---- main loop: per r-chunk, per m-tile: S pair -> exp ->
            # O accumulate; V_aug built lazily on the first r-chunk --------
            QT = consts.tile([128, R_PAD], bf16, tag="QT")
            OnT = consts.tile([128, R_PAD], bf16, tag="OnT")
            # y r-tile groups: consecutive FULL tiles pair up; a partial
            # trailing tile is emitted alone.
            n_full = R_PAD // 128
            YG = [(i, i + 1) for i in range(0, n_full - 1, 2)]
            if n_full % 2 == 1:
                YG.append((n_full - 1,))
            if R_PAD % 128 != 0:
                YG.append((n_full,))
            ydone = 0

            def q_proj(ci):
                o, w = RC[ci]
                qps = psQ.tile([128, 2, 512], f32, tag="Q", name=f"qps{ci}")
                for c in range(4):
                    nc.tensor.matmul(qps[:, 0, :w], wq[:, c, :],
                                     xT[:, c, o:o + w],
                                     start=(c == 0), stop=(c == 3))
                if ci == 0:
                    nc.scalar.copy(QT[:, o:o + w], qps[:, 0, :w])
                else:
                    nc.vector.tensor_copy(QT[:, o:o + w], qps[:, 0, :w])

            q_proj(0)
            for ci, (o, w) in enumerate(RC):
                op = psO.tile([128, 2, 512], f32, tag="O", name=f"op{ci}")
                pts = {}
                for mt in range(NMT):
                    msl = slice(mt * 128, (mt + 1) * 128)
                    if ci == 0:
                        # lazy V_aug into Vt[:, mt]: [kvc x64 | V_h x64] per
                        # head; the kvc columns (DMA'd from host) make the O
                        # matmul emit the softmax denominator replicated on
                        # partitions 0:64 and null the tail rows sitting
                        # below M_PAD. The V matmul is split per head so
                        # each head's V columns land in separate PSUM banks,
                        # letting one DVE op scale-copy both heads at once.
                        vps = psQ.tile([128, 2, 512], f32, tag="Q",
                                       name=f"vps{mt}")
                        for c in range(4):
                            for h in range(2):
                                nc.tensor.matmul(
                                    vps[:, h, 0:64], xT[:, c, msl],
                                    wv[:, c, h * 64:(h + 1) * 64],
                                    start=(c == 0), stop=(c == 3))
                        nc.vector.tensor_scalar_mul(
                            Vt[:, mt, :, 64:128], in0=vps[:, :, 0:64],
                            scalar1=kvc[:, mt:mt + 1])
                    sp = psS.tile([128, 2, 512], f32, tag="S",
                                  name=f"sp{ci}_{mt}")
                    for h in range(2):
                        hs = slice(h * 64, (h + 1) * 64)
                        nc.tensor.matmul(sp[:, h, :w], KT[hs, msl],
                                         QT[hs, o:o + w],
                                         start=True, stop=True)
                    pt = ptpool.tile([128, 2, 512], bf16, tag="pt",
                                     name=f"pt{ci}_{mt}")
                    nc.scalar.activation(out=pt[:, :, :w], in_=sp[:, :, :w],
                                         func=EXP)
                    pts[mt] = pt
                    # O for the PREVIOUS m-tile: keeps the PE busy on
                    # S(mt) while ScalarE runs exp(mt-1) instead of the
                    # in-order PE queue stalling on exp(mt).
                    if mt > 0:
                        for h in range(2):
                            nc.tensor.matmul(op[:, h, :w],
                                             Vt[:, mt - 1, h, :],
                                             pts[mt - 1][:, h, :w],
                                             start=(mt == 1), stop=False)
                for h in range(2):
                    nc.tensor.matmul(op[:, h, :w], Vt[:, NMT - 1, h, :],
                                     pts[NMT - 1][:, h, :w],
                                     start=False, stop=True)

                # normalize: denominators sit replicated on partitions 0:64
                rc = rpool.tile([64, 2, 512], f32, tag="rcp", name=f"rc{ci}")
                nc.vector.reciprocal_approx_fast(rc[:, :, :w],
                                                 op[0:64, :, :w])
                for h in range(2):
                    nc.vector.tensor_mul(OnT[h * 64:(h + 1) * 64, o:o + w],
                                         op[64:128, h, :w], rc[:, h, :w])

                # next r-chunk's Q projection BEFORE this chunk's y tiles:
                # keeps the psQ slot rotation free of cross-ci stalls.
                if ci + 1 < len(RC):
                    q_proj(ci + 1)

                # out projection for r-tile groups fully covered so far;
                # two full r-tiles share one 2-bank PSUM tile so a single
                # copy + DMA moves both (copies alternate DVE / ScalarE).
                done = o + w
                while ydone < len(YG):
                    rts = YG[ydone]
                    lo = rts[0] * 128
                    hi = min(rts[-1] * 128 + 128, R_PAD)
                    if hi > done:
                        break
                    yp = psQ.tile([128, 2, 512], f32, tag="Q",
                                  name=f"yp{rts[0]}")
                    for j, rt in enumerate(rts):
                        tw = min(128, R_PAD - rt * 128)
                        nc.tensor.matmul(
                            yp[:tw, j, :], OnT[:, rt * 128:rt * 128 + tw],
                            wo, start=True, stop=True)
                    copy = (nc.vector.tensor_copy if ydone % 2 == 0
                            else nc.scalar.copy)
                    deng = [nc.sync, nc.scalar, nc.gpsimd][ydone % 3]
                    ysb = ypool.tile([128, 2, 512], bf16, tag="y")
                    if len(rts) == 2:
                        copy(ysb, yp)
                        deng.dma_start(
                            out=y_d.ap()[lo:hi, :].rearrange(
                                "(a p) d -> p a d", p=128),
                            in_=ysb)
                    else:
                        copy(ysb[:hi - lo, 0, :], yp[:hi - lo, 0, :])
                        deng.dma_start(out=y_d.ap()[lo:hi, :],
                                       in_=ysb[:hi - lo, 0, :])
                    ydone += 1

    nc.compile()
    return nc


def _prep(x, mask_np, mask_bert, W_qkv, W_out):
    """Host-side gather/shard. Returns (in_maps, meta)."""
    import ml_dtypes
    bf16 = ml_dtypes.bfloat16

    B, N, DIM = x.shape
    assert (B, DIM) == (2, _DIM)
    x = np.ascontiguousarray(x, dtype=np.float32)
    W_qkv = np.ascontiguousarray(W_qkv, dtype=np.float32)
    W_out = np.ascontiguousarray(W_out, dtype=np.float32)

    kv_idx, tail_idx, Ms, tails = [], [], [], []
    for b in range(B):
        npb = mask_np[b].astype(bool)
        bb = mask_bert[b].astype(bool)
        kv = np.nonzero(npb & ~bb)[0]
        tl = np.nonzero(npb & bb)[0]
        kv_idx.append(kv)
        tail_idx.append(tl)
        Ms.append(len(kv))
        tails.append(len(tl))

    M_PAD = max(128, _ceil_to(max(Ms), 128))
    # rows are packed [kv | tail] with no gap: the tail rows that fall in
    # [M_b, M_PAD) act as key/value candidates but are nulled by the kvc
    # indicator (V rows scaled to 0, denominator columns 0).
    R_PAD = max(128, _ceil_to(max(Ms[b] + tails[b] for b in range(B)), 16),
                M_PAD)

    NMT = M_PAD // 128
    xT_b, kvc_b, kvr_b, row_pos = [], [], [], []
    for b in range(B):
        xa = np.zeros((512, R_PAD), dtype=np.float32)
        xa[:, :Ms[b]] = x[b][kv_idx[b]].T
        xa[:, Ms[b]:Ms[b] + tails[b]] = x[b][tail_idx[b]].T
        xT_b.append(np.ascontiguousarray(xa.astype(bf16)))
        kvones = np.zeros(M_PAD, dtype=np.float32)
        kvones[:Ms[b]] = 1.0
        kvc_b.append(np.ascontiguousarray(kvones.reshape(NMT, 128).T))
        # kvc replicated into the 64 denominator columns per (m-tile, head)
        kvr = np.broadcast_to(kvc_b[b][:, :, None, None],
                              (128, NMT, 2, 64))
        kvr_b.append(np.ascontiguousarray(
            kvr.reshape(128, NMT * 128).astype(bf16)))
        # output row p of the device result corresponds to token row_pos[p]
        pos = np.concatenate([kv_idx[b], tail_idx[b]])
        row_pos.append(pos)

    scale = np.float32(_DH ** -0.5)
    in_maps = []
    for c in range(_CORES):
        b, g = divmod(c, 4)
        qc = slice(128 * g, 128 * g + 128)
        kc = slice(_INNER + 128 * g, _INNER + 128 * g + 128)
        vc = slice(2 * _INNER + 128 * g, 2 * _INNER + 128 * g + 128)
        wq = np.ascontiguousarray((W_qkv[:, qc] * scale).astype(bf16))
        wk = np.ascontiguousarray(W_qkv[:, kc].astype(bf16))
        wv = np.ascontiguousarray(W_qkv[:, vc].astype(bf16))
        wo = np.ascontiguousarray(
            W_out[128 * g:128 * g + 128, :].astype(bf16))
        in_maps.append({"xT": xT_b[b], "wq": wq, "wk": wk, "wv": wv, "wo": wo,
                        "kvc": kvc_b[b], "kvr": kvr_b[b]})

    meta = dict(M_PAD=M_PAD, R_PAD=R_PAD, Ms=Ms, tails=tails,
                kv_idx=kv_idx, tail_idx=tail_idx, row_pos=row_pos)
    return in_maps, meta


def _assemble(results, meta, x, mask_np, W_qkv, W_out, b_out):
    B, N, _ = x.shape
    out = np.empty((B, N, _DIM), dtype=np.float32)
    Wv_full = W_qkv[:, 2 * _INNER:].astype(np.float32)
    for b in range(B):
        # constant output for fully-masked rows: uniform attention = mean(V)
        meanv = (x[b].mean(axis=0, dtype=np.float32) @ Wv_full)
        yconst = meanv @ W_out.astype(np.float32) + b_out
        out[b, :, :] = yconst[None, :]
        Mb, tb = meta["Ms"][b], meta["tails"][b]
        if Mb == 0:
            # no unmasked kv columns: every row fully masked -> uniform
            continue
        acc = None
        for g in range(4):
            yp = results[4 * b + g]["y"].astype(np.float32)
            acc = yp if acc is None else acc + yp
        out[b, meta["row_pos"][b], :] = acc[:Mb + tb] + b_out
    return out


_CACHE = {}


def _get_bass(R_PAD, M_PAD):
    key = (R_PAD, M_PAD)
    if key not in _CACHE:
        _CACHE[key] = build_bass(R_PAD, M_PAD)
    return _CACHE[key]


def run_spmd(in_maps, meta, trace=False, tmpdir=None, trace_cores=None):
    from concourse.bass_utils import run_bass_kernel_spmd

    nc = _get_bass(meta["R_PAD"], meta["M_PAD"])
    return run_bass_kernel_spmd(
        nc, in_maps, core_ids=list(range(_CORES)), trace=trace, tmpdir=tmpdir,
        trace_cores=trace_cores)


def kernel(x, mask_np, mask_bert, W_qkv, W_out, b_out):
    x = np.asarray(x)
    mask_np = np.asarray(mask_np)
    mask_bert = np.asarray(mask_bert)
    W_qkv = np.asarray(W_qkv, dtype=np.float32)
    W_out = np.asarray(W_out, dtype=np.float32)
    b_out = np.asarray(b_out, dtype=np.float32)

    in_maps, meta = _prep(x, mask_np, mask_bert, W_qkv, W_out)
    res = run_spmd(in_maps, meta)
    return _assemble(res.results, meta, x, mask_np, W_qkv, W_out, b_out)


# revision 36
# speedup vs baseline: 2.0077x; 1.0270x over previous
"""Sparse dual-masked attention for Trainium2, 8 NeuronCores.

Problem: B=2, N=2048, DIM=512, H=8, DH=64.
  qkv = x @ W_qkv; per-head attention with dual mask
  (np_i*np_j==0 | bert_j==1 -> -1000), softmax, out proj + bias.

Structure exploited (sparse_attention):
  - A row i with np_i==0 is fully masked -> softmax uniform -> output row is
    the constant mean(V) @ W_out + b_out (computed on host; tiny).
  - For np_i==1 rows only columns with np_j==1 & bert_j==0 survive, so we
    gather those ~R=1030 rows / ~M=535 kv columns on the host and run dense
    attention over the gathered set on device (~8x less work than dense).

Sharding: core = (batch b, head-pair g): 2 batches x 4 head groups.
  W_qkv split column-wise per head pair, W_out row-wise; each core produces
  a partial [R,512] output; host sums the 4 partials per batch.

All matmul operands are bf16 (PSUM accumulation stays fp32): 1 cyc/row on
the PE at any free size vs 4 for fp32, half DMA/SBUF/LDWEIGHTS cost. A host
simulation of 8-bit-mantissa rounding through the whole pipeline gives
~2.6e-3 scale-relative error vs the 2e-2 gate.

Device dataflow per core (R_PAD query rows, M_PAD kv cols, 2 heads):
  xT [512, R_PAD] ships pre-gathered/transposed (kv rows first, then tail
  rows); kvc [128, NMT] is the kv-indicator column per m-tile.
  1. K^T = Wk^T x^T [128, M_PAD]; Q^T chunks computed lazily per r-chunk.
  2. Per r-chunk (PSUM-bank-sized, <=512), per m-tile: V_aug built lazily on
     the first pass: V rows scaled by kvc, plus kvc replicated into 64
     stationary columns per head so the attn@V matmul emits the softmax
     denominator REPLICATED on psum partitions 0:64 for free.
     S^T pair (both heads, disjoint PE row groups) -> one exp (ScalarE,
     2-bank PSUM AP -> bf16 SBUF) -> O^T += V_aug_h^T P_h^T accumulated
     over m-tiles into a 2-bank pair.
  3. recip = 1/denoms on the 64 replicated partitions directly (DVE approx),
     O^T normalized by tensor_mul -> OnT bf16; y = OnT^T @ W_out rows per
     128-r-tile as soon as both heads are normalized; DMA out interleaved.
  Host adds bias, sums the 4 head-group partials, fills masked rows.
"""

import numpy as np

_CORES = 8
_DIM = 512
_DH = 64
_H = 8
_INNER = _H * _DH


def _ceil_to(x, m):
    return ((x + m - 1) // m) * m


def _chunks(total, step):
    out = []
    o = 0
    while o < total:
        out.append((o, min(step, total - o)))
        o += step
    return out


def _chunks_ge(total, step=512, minc=256):
    """Chunks of <= step, each >= minc (rebalancing the tail)."""
    out = _chunks(total, step)
    if len(out) >= 2 and out[-1][1] < minc:
        o_prev, w_prev = out[-2]
        o_last, w_last = out[-1]
        move = minc - w_last
        out[-2] = (o_prev, w_prev - move)
        out[-1] = (o_last - move, w_last + move)
    return out


def build_bass(R_PAD, M_PAD):
    """Build the SPMD bass program for padded sizes R_PAD (queries) and
    M_PAD (kv columns). Returns the compiled Bacc object."""
    import concourse.bacc as bacc
    import concourse.mybir as mybir
    import concourse.tile as tile

    f32 = mybir.dt.float32
    bf16 = mybir.dt.bfloat16
    EXP = mybir.ActivationFunctionType.Exp

    assert R_PAD % 16 == 0 and M_PAD % 128 == 0 and R_PAD >= M_PAD
    NMT = M_PAD // 128          # kv m-tiles
    NRT = (R_PAD + 127) // 128  # query r-tiles for the final projection
    RC = _chunks_ge(R_PAD)      # r-chunks: one PSUM bank each, >=256
    MC = _chunks_ge(M_PAD)
    assert len(RC) <= 3

    nc = bacc.Bacc("TRN2", target_bir_lowering=False, debug=False,
                   num_devices=_CORES)

    xT_d = nc.dram_tensor("xT", [512, R_PAD], bf16, kind="ExternalInput")
    w3_d = nc.dram_tensor("w3", [512, 384], bf16, kind="ExternalInput")
    kvc_d = nc.dram_tensor("kvc", [128, NMT], f32, kind="ExternalInput")
    kvr_d = nc.dram_tensor("kvr", [128, NMT * 128], bf16,
                           kind="ExternalInput")
    wo_d = nc.dram_tensor("wo", [128, 512], bf16, kind="ExternalInput")
    y_d = nc.dram_tensor("y", [R_PAD, 512], bf16, kind="ExternalOutput")

    with tile.TileContext(nc) as tc:
        with (
            tc.tile_pool(name="consts", bufs=1) as consts,
            tc.tile_pool(name="pt", bufs=4) as ptpool,
            tc.tile_pool(name="rcp", bufs=2) as rpool,
            tc.tile_pool(name="ysb", bufs=4) as ypool,
            tc.tile_pool(name="psS", bufs=2, space="PSUM") as psS,
            tc.tile_pool(name="psO", bufs=1, space="PSUM") as psO,
            tc.tile_pool(name="psQ", bufs=1, space="PSUM") as psQ,
        ):
            # ---- input DMAs: merged to 8 issues over the 3 DMA queues;
            # the critical first wave (all weights + xT's kv column
            # prefix) is balanced one-per-queue --------------------------
            w3 = consts.tile([128, 4, 384], bf16, tag="w3")
            nc.scalar.dma_start(
                out=w3, in_=w3_d.ap().rearrange("(a p) d -> p a d", p=128))
            xT = consts.tile([128, 4, R_PAD], bf16, tag="xT")
            Vt = consts.tile([128, NMT, 2, 128], bf16, tag="Vt")
            for cp, eng in ((0, nc.sync), (1, nc.gpsimd)):
                nc_sl = slice(cp * 256, cp * 256 + 256)
                eng.dma_start(
                    out=xT[:, 2 * cp:2 * cp + 2, 0:M_PAD],
                    in_=xT_d.ap()[nc_sl, 0:M_PAD].rearrange(
                        "(a p) d -> p a d", p=128))
            kvc = consts.tile([128, NMT], f32, tag="kvc")
            wo = consts.tile([128, 512], bf16, tag="wo")
            for cp, eng in ((0, nc.sync), (1, nc.gpsimd)):
                nc_sl = slice(cp * 256, cp * 256 + 256)
                eng.dma_start(
                    out=xT[:, 2 * cp:2 * cp + 2, M_PAD:R_PAD],
                    in_=xT_d.ap()[nc_sl, M_PAD:R_PAD].rearrange(
                        "(a p) d -> p a d", p=128))
            nc.scalar.dma_start(out=kvc, in_=kvc_d.ap())
            nc.scalar.dma_start(
                out=Vt[:, :, :, 0:64],
                in_=kvr_d.ap().rearrange("p (a b c) -> p a b c", b=2, c=64))
            nc.scalar.dma_start(out=wo, in_=wo_d.ap())
            def wq(c):
                return w3[:, c, 0:128]

            def wk(c):
                return w3[:, c, 128:256]

            def wv(c, h):
                return w3[:, c, 256 + h * 64:256 + (h + 1) * 64]

            # ---- K projection (Q is computed lazily per r-chunk below) --
            KT = consts.tile([128, M_PAD], bf16, tag="KT")
            for i, (o, w) in enumerate(MC):
                ps = psO.tile([128, 2, 512], f32, tag="O", name=f"kps{i}")
                for c in range(4):
                    nc.tensor.matmul(ps[:, 0, :w], wk(c),
                                     xT[:, c, o:o + w],
                                     start=(c == 0), stop=(c == 3))
                nc.scalar.copy(KT[:, o:o + w], ps[:, 0, :w])

            # ---- main loop: per r-chunk, per m-tile: S pair -> exp ->
            # O accumulate; V_aug built lazily on the first r-chunk --------
            QT = consts.tile([128, R_PAD], bf16, tag="QT")
            OnT = consts.tile([128, R_PAD], bf16, tag="OnT")
            # y r-tile groups: consecutive FULL tiles pair up; a partial
            # trailing tile is emitted alone.
            n_full = R_PAD // 128
            YG = [(i, i + 1) for i in range(0, n_full - 1, 2)]
            if n_full % 2 == 1:
                YG.append((n_full - 1,))
            if R_PAD % 128 != 0:
                YG.append((n_full,))
            ydone = 0

            def q_proj(ci):
                o, w = RC[ci]
                qps = psQ.tile([128, 2, 512], f32, tag="Q", name=f"qps{ci}")
                for c in range(4):
                    nc.tensor.matmul(qps[:, 0, :w], wq(c),
                                     xT[:, c, o:o + w],
                                     start=(c == 0), stop=(c == 3))
                nc.vector.tensor_copy(QT[:, o:o + w], qps[:, 0, :w])

            def emit_y(rts, k):
                lo = rts[0] * 128
                hi = min(rts[-1] * 128 + 128, R_PAD)
                yp = psQ.tile([128, 2, 512], f32, tag="Q",
                              name=f"yp{rts[0]}")
                for j, rt in enumerate(rts):
                    tw = min(128, R_PAD - rt * 128)
                    nc.tensor.matmul(
                        yp[:tw, j, :], OnT[:, rt * 128:rt * 128 + tw],
                        wo, start=True, stop=True)
                copy = nc.vector.tensor_copy if k % 2 == 0 else nc.scalar.copy
                deng = [nc.sync, nc.scalar, nc.gpsimd][k % 3]
                ysb = ypool.tile([128, 2, 512], bf16, tag="y")
                if len(rts) == 2:
                    copy(ysb, yp)
                    deng.dma_start(
                        out=y_d.ap()[lo:hi, :].rearrange(
                            "(a p) d -> p a d", p=128),
                        in_=ysb)
                else:
                    copy(ysb[:hi - lo, 0, :], yp[:hi - lo, 0, :])
                    deng.dma_start(out=y_d.ap()[lo:hi, :],
                                   in_=ysb[:hi - lo, 0, :])

            q_proj(0)
            pending = []
            nemit = 0
            for ci, (o, w) in enumerate(RC):
                op = psO.tile([128, 2, 512], f32, tag="O", name=f"op{ci}")
                pts = {}
                for mt in range(NMT):
                    msl = slice(mt * 128, (mt + 1) * 128)
                    if mt >= 3 and pending:
                        # deferred y group from the previous r-chunk: by
                        # now its OnT muls are long done, so the PE queue
                        # doesn't stall on them.
                        emit_y(pending.pop(0), nemit)
                        nemit += 1
                    if mt == NMT - 2 and ci + 1 < len(RC):
                        # next r-chunk's Q projection, early enough that
                        # its QT copy clears the DVE queue before this
                        # chunk's rcp/muls pile in.
                        q_proj(ci + 1)
                    if ci == 0:
                        # lazy V_aug into Vt[:, mt]: [kvc x64 | V_h x64] per
                        # head; the kvc columns (DMA'd from host) make the O
                        # matmul emit the softmax denominator replicated on
                        # partitions 0:64 and null the tail rows sitting
                        # below M_PAD. The V matmul is split per head so
                        # each head's V columns land in separate PSUM banks,
                        # letting one DVE op scale-copy both heads at once.
                        vps = psQ.tile([128, 2, 512], f32, tag="Q",
                                       name=f"vps{mt}")
                        for c in range(4):
                            for h in range(2):
                                nc.tensor.matmul(
                                    vps[:, h, 0:64], xT[:, c, msl],
                                    wv(c, h),
                                    start=(c == 0), stop=(c == 3))
                        nc.vector.tensor_scalar_mul(
                            Vt[:, mt, :, 64:128], in0=vps[:, :, 0:64],
                            scalar1=kvc[:, mt:mt + 1])
                    sp = psS.tile([128, 2, 512], f32, tag="S",
                                  name=f"sp{ci}_{mt}")
                    for h in range(2):
                        hs = slice(h * 64, (h + 1) * 64)
                        nc.tensor.matmul(sp[:, h, :w], KT[hs, msl],
                                         QT[hs, o:o + w],
                                         start=True, stop=True)
                    pt = ptpool.tile([128, 2, 512], bf16, tag="pt",
                                     name=f"pt{ci}_{mt}")
                    nc.scalar.activation(out=pt[:, :, :w], in_=sp[:, :, :w],
                                         func=EXP)
                    pts[mt] = pt
                    # O for the PREVIOUS m-tile: keeps the PE busy on
                    # S(mt) while ScalarE runs exp(mt-1) instead of the
                    # in-order PE queue stalling on exp(mt).
                    if mt > 0:
                        for h in range(2):
                            nc.tensor.matmul(op[:, h, :w],
                                             Vt[:, mt - 1, h, :],
                                             pts[mt - 1][:, h, :w],
                                             start=(mt == 1), stop=False)
                for h in range(2):
                    nc.tensor.matmul(op[:, h, :w], Vt[:, NMT - 1, h, :],
                                     pts[NMT - 1][:, h, :w],
                                     start=False, stop=True)

                # normalize: denominators sit replicated on partitions 0:64
                rc = rpool.tile([64, 2, 512], f32, tag="rcp", name=f"rc{ci}")
                nc.vector.reciprocal_approx_fast(rc[:, :, :w],
                                                 op[0:64, :, :w])
                for h in range(2):
                    nc.vector.tensor_mul(OnT[h * 64:(h + 1) * 64, o:o + w],
                                         op[64:128, h, :w], rc[:, h, :w])

                # queue y r-tile groups now fully covered; they are
                # emitted a few m-tiles into the NEXT r-chunk (or flushed
                # at the end) so the PE queue never stalls on the muls.
                done = o + w
                while (ydone < len(YG) and
                       min(YG[ydone][-1] * 128 + 128, R_PAD) <= done):
                    pending.append(YG[ydone])
                    ydone += 1
            for rts in pending:
                emit_y(rts, nemit)
                nemit += 1

    nc.compile()
    return nc


def _prep(x, mask_np, mask_bert, W_qkv, W_out):
    """Host-side gather/shard. Returns (in_maps, meta)."""
    import ml_dtypes
    bf16 = ml_dtypes.bfloat16

    B, N, DIM = x.shape
    assert (B, DIM) == (2, _DIM)
    x = np.ascontiguousarray(x, dtype=np.float32)
    W_qkv = np.ascontiguousarray(W_qkv, dtype=np.float32)
    W_out = np.ascontiguousarray(W_out, dtype=np.float32)

    kv_idx, tail_idx, Ms, tails = [], [], [], []
    for b in range(B):
        npb = mask_np[b].astype(bool)
        bb = mask_bert[b].astype(bool)
        kv = np.nonzero(npb & ~bb)[0]
        tl = np.nonzero(npb & bb)[0]
        kv_idx.append(kv)
        tail_idx.append(tl)
        Ms.append(len(kv))
        tails.append(len(tl))

    M_PAD = max(128, _ceil_to(max(Ms), 128))
    # rows are packed [kv | tail] with no gap: the tail rows that fall in
    # [M_b, M_PAD) act as key/value candidates but are nulled by the kvc
    # indicator (V rows scaled to 0, denominator columns 0).
    R_PAD = max(128, _ceil_to(max(Ms[b] + tails[b] for b in range(B)), 16),
                M_PAD)

    NMT = M_PAD // 128
    xT_b, kvc_b, kvr_b, row_pos = [], [], [], []
    for b in range(B):
        xa = np.zeros((512, R_PAD), dtype=np.float32)
        xa[:, :Ms[b]] = x[b][kv_idx[b]].T
        xa[:, Ms[b]:Ms[b] + tails[b]] = x[b][tail_idx[b]].T
        xT_b.append(np.ascontiguousarray(xa.astype(bf16)))
        kvones = np.zeros(M_PAD, dtype=np.float32)
        kvones[:Ms[b]] = 1.0
        kvc_b.append(np.ascontiguousarray(kvones.reshape(NMT, 128).T))
        # kvc replicated into the 64 denominator columns per (m-tile, head)
        kvr = np.broadcast_to(kvc_b[b][:, :, None, None],
                              (128, NMT, 2, 64))
        kvr_b.append(np.ascontiguousarray(
            kvr.reshape(128, NMT * 128).astype(bf16)))
        # output row p of the device result corresponds to token row_pos[p]
        pos = np.concatenate([kv_idx[b], tail_idx[b]])
        row_pos.append(pos)

    scale = np.float32(_DH ** -0.5)
    in_maps = []
    for c in range(_CORES):
        b, g = divmod(c, 4)
        qc = slice(128 * g, 128 * g + 128)
        kc = slice(_INNER + 128 * g, _INNER + 128 * g + 128)
        vc = slice(2 * _INNER + 128 * g, 2 * _INNER + 128 * g + 128)
        w3 = np.ascontiguousarray(np.concatenate(
            [W_qkv[:, qc] * scale, W_qkv[:, kc], W_qkv[:, vc]],
            axis=1).astype(bf16))
        wo = np.ascontiguousarray(
            W_out[128 * g:128 * g + 128, :].astype(bf16))
        in_maps.append({"xT": xT_b[b], "w3": w3, "wo": wo,
                        "kvc": kvc_b[b], "kvr": kvr_b[b]})

    meta = dict(M_PAD=M_PAD, R_PAD=R_PAD, Ms=Ms, tails=tails,
                kv_idx=kv_idx, tail_idx=tail_idx, row_pos=row_pos)
    return in_maps, meta


def _assemble(results, meta, x, mask_np, W_qkv, W_out, b_out):
    B, N, _ = x.shape
    out = np.empty((B, N, _DIM), dtype=np.float32)
    Wv_full = W_qkv[:, 2 * _INNER:].astype(np.float32)
    for b in range(B):
        # constant output for fully-masked rows: uniform attention = mean(V)
        meanv = (x[b].mean(axis=0, dtype=np.float32) @ Wv_full)
        yconst = meanv @ W_out.astype(np.float32) + b_out
        out[b, :, :] = yconst[None, :]
        Mb, tb = meta["Ms"][b], meta["tails"][b]
        if Mb == 0:
            # no unmasked kv columns: every row fully masked -> uniform
            continue
        acc = None
        for g in range(4):
            yp = results[4 * b + g]["y"].astype(np.float32)
            acc = yp if acc is None else acc + yp
        out[b, meta["row_pos"][b], :] = acc[:Mb + tb] + b_out
    return out


_CACHE = {}


def _get_bass(R_PAD, M_PAD):
    key = (R_PAD, M_PAD)
    if key not in _CACHE:
        _CACHE[key] = build_bass(R_PAD, M_PAD)
    return _CACHE[key]


def run_spmd(in_maps, meta, trace=False, tmpdir=None, trace_cores=None):
    from concourse.bass_utils import run_bass_kernel_spmd

    nc = _get_bass(meta["R_PAD"], meta["M_PAD"])
    return run_bass_kernel_spmd(
        nc, in_maps, core_ids=list(range(_CORES)), trace=trace, tmpdir=tmpdir,
        trace_cores=trace_cores)


def kernel(x, mask_np, mask_bert, W_qkv, W_out, b_out):
    x = np.asarray(x)
    mask_np = np.asarray(mask_np)
    mask_bert = np.asarray(mask_bert)
    W_qkv = np.asarray(W_qkv, dtype=np.float32)
    W_out = np.asarray(W_out, dtype=np.float32)
    b_out = np.asarray(b_out, dtype=np.float32)

    in_maps, meta = _prep(x, mask_np, mask_bert, W_qkv, W_out)
    res = run_spmd(in_maps, meta)
    return _assemble(res.results, meta, x, mask_np, W_qkv, W_out, b_out)
